# revision 8
# baseline (speedup 1.0000x reference)
"""Trainium2 Bass kernel for nn_ContextGatingSigmoidClassifier.

Math (eval mode):
  f_so = lrelu(W_so @ bn_so(x_so) + b_so)        x: [B,2048,N,H,W]
  f_c  = lrelu(W_c  @ bn_c(x_c)  + b_c)
  f    = concat -> bn1 -> W1 -> bn2 -> lrelu -> W2 -> mean(H,W) -> sigmoid > 0.5

All BatchNorms are eval-mode affine maps, so they fold into the adjacent
linear layers (done host-side in fp64):
  A_so = W_so * s_so ; a_so = W_so @ t_so + b_so          (s,t: bn scale/shift)
  A1   = diag(s2) W1 diag(s1) ; a1 = s2*(W1 @ t1 + b1) + t2
Final threshold: sigmoid(mean) > 0.5  <=>  sum_hw(W2 @ h) > -49*b2.

Device mapping: data-parallel over batch (4 per core, 8 cores). Weights
replicated. Per core, per batch element b:
  x[b] is [2048, 588] (channels x positions). Matmuls keep channels on
  SBUF partitions (K-chunks of 128), positions on the free dim (2 tiles
  of 294 = one PSUM bank each). All matmul operands fp16 (PSUM accum is
  fp32); fp32->fp16 happens inside the input DMA (SWDGE cast).
  Channel->partition mapping for x is interleaved (partition p holds
  channels 16p..16p+15) so each DMA descriptor reads contiguous runs;
  layer-1 weights are permuted host-side to match.

Schedule: one ordered gpsimd DMA chain delivers weights and x chunks in
exactly the order the PE consumes them. Layer-1 accumulates k-sub-outer
across all 8 PSUM tiles of a branch, so compute starts after the first
x sub-DMA instead of after the whole tensor; both m-halves share each
loaded weight tile back to back. Batches 0+1 run their so-branches
before any c-branch so the DMA-critical prefix is wso + two x_so
tensors (wc defers ~2 branch-times). The final layer reduces h over
the 49 HW positions first (DVE, per m-half as its activations land),
then contracts 256 channels with a tiny fp32 matmul; each batch's
final contraction is deferred past the next batch's matmuls so the
in-order PE never stalls on the activation->reduce chain, and only the
last batch's chain sits on the kernel tail.
"""

import numpy as np

import concourse.bass as bass  # noqa: F401  (engine types referenced via nc)
import concourse.tile as tile
from concourse import bacc, mybir
from concourse.bass_utils import run_bass_kernel_spmd

F16 = mybir.dt.float16
F32 = mybir.dt.float32

B, C, NN, HW = 32, 2048, 12, 49
NHW = NN * HW            # 588
N_CORES = 8
BPC = B // N_CORES       # 4 batch elements per core
MT = NHW // 2            # 294 columns = one PSUM bank of fp32
KC1 = C // 128           # 16 K-chunks, layer 1
OC1 = 512 // 128         # 4 output chunks, layer 1 (per branch)
KC2 = 1024 // 128        # 8 K-chunks, layer 2
OC2 = 256 // 128         # 2 output chunks, layer 2
EPS = 1e-5
SLOPE = 0.2


def _fold_params(d):
    """Fold BNs into linears, in fp64. Returns device-layout arrays."""
    g = {k: np.asarray(v, dtype=np.float64) for k, v in d.items()}

    def bn_st(p):
        s = g[f"{p}_g"] / np.sqrt(g[f"{p}_v"] + EPS)
        t = g[f"{p}_b"] - g[f"{p}_m"] * s
        return s, t

    s_so, t_so = bn_st("bn_so")
    s_c, t_c = bn_st("bn_c")
    s1, t1 = bn_st("bn1")
    s2, t2 = bn_st("bn2")

    A_so = g["W_so"] * s_so[None, :]                 # [512, 2048]
    a_so = g["W_so"] @ t_so + g["b_so"]              # [512]
    A_c = g["W_c"] * s_c[None, :]
    a_c = g["W_c"] @ t_c + g["b_c"]
    A1 = s2[:, None] * (g["W1"] * s1[None, :])       # [256, 1024]
    a1 = s2 * (g["W1"] @ t1 + g["b1"]) + t2          # [256]

    # layer-1 weights: chunk j holds channel 16p+j at partition p (matches
    # the contiguous-run x DMA layout). Stored p-major [128, k*m] so each
    # DMA descriptor is one contiguous per-partition run.
    def l1_prep(A):  # [512, 2048] -> [128, 16*512] fp16
        AT = A.T.reshape(128, 16, 512)               # [p, j, m] with ch = 16p+j
        return np.ascontiguousarray(AT.reshape(128, KC1 * 512)).astype(np.float16)

    wso = l1_prep(A_so)
    wc = l1_prep(A_c)
    w1 = np.ascontiguousarray(
        A1.T.reshape(KC2, 128, 256).transpose(1, 0, 2).reshape(128, KC2 * 256)
    ).astype(np.float16)
    w2 = np.ascontiguousarray(g["W2"].reshape(OC2, 128).T).astype(np.float32)  # [128, 2]
    bso = np.ascontiguousarray(a_so.reshape(OC1, 128).T).astype(np.float32)    # [128, 4]
    bc = np.ascontiguousarray(a_c.reshape(OC1, 128).T).astype(np.float32)
    b1 = np.ascontiguousarray(a1.reshape(OC2, 128).T).astype(np.float32)       # [128, 2]
    thresh = float(-HW * g["b2"][0])
    return wso, wc, w1, w2, bso, bc, b1, thresh


def build_bass(thresh, loop=1):
    """loop: on-device For_i wrapper around the whole body (timing only)."""
    nc = bacc.Bacc("TRN2", target_bir_lowering=False, debug=False)

    xso_d = nc.dram_tensor("x_so", [BPC, C, NHW], F32, kind="ExternalInput").ap()
    xc_d = nc.dram_tensor("x_c", [BPC, C, NHW], F32, kind="ExternalInput").ap()
    wso_d = nc.dram_tensor("wso", [128, KC1 * 512], F16, kind="ExternalInput").ap()
    wc_d = nc.dram_tensor("wc", [128, KC1 * 512], F16, kind="ExternalInput").ap()
    w1_d = nc.dram_tensor("w1", [128, KC2 * 256], F16, kind="ExternalInput").ap()
    w2_d = nc.dram_tensor("w2", [128, OC2], F32, kind="ExternalInput").ap()
    bso_d = nc.dram_tensor("bso", [128, OC1], F32, kind="ExternalInput").ap()
    bc_d = nc.dram_tensor("bc", [128, OC1], F32, kind="ExternalInput").ap()
    b1_d = nc.dram_tensor("b1", [128, OC2], F32, kind="ExternalInput").ap()
    out_d = nc.dram_tensor("out", [BPC * NN], F32, kind="ExternalOutput").ap()

    with tile.TileContext(nc) as tc:
        with (
            tc.tile_pool(name="wp", bufs=1) as wp,
            tc.tile_pool(name="xp", bufs=2) as xp,
            tc.tile_pool(name="fp", bufs=2) as fp,
            tc.tile_pool(name="hp", bufs=2) as hp,
            tc.tile_pool(name="mp", bufs=4) as mp,
            tc.tile_pool(name="ap", bufs=1) as ac,
            tc.tile_pool(name="ps", bufs=8, space="PSUM") as ps,
        ):
            # ---- biases / small tensors on the HWDGE (sync) ring ----
            bso_sb = wp.tile([128, OC1], F32)
            nc.sync.dma_start(bso_sb[:], bso_d[:])
            bc_sb = wp.tile([128, OC1], F32)
            nc.sync.dma_start(bc_sb[:], bc_d[:])
            b1_sb = wp.tile([128, OC2], F32)
            nc.sync.dma_start(b1_sb[:], b1_d[:])
            w2_sb = wp.tile([128, OC2], F32)
            nc.sync.dma_start(w2_sb[:], w2_d[:])
            # big weights ride the ordered gpsimd chain (see _body),
            # interleaved with the x stream at their consumption points.
            wso_sb = wp.tile([128, KC1 * 512], F16)
            wc_sb = wp.tile([128, KC1 * 512], F16)
            w1_sb = wp.tile([128, KC2 * 256], F16)

            bits_sb = ac.tile([1, BPC * NN], F32)

            import contextlib
            loop_cm = tc.For_i(0, loop, 1) if loop > 1 else contextlib.nullcontext()
            with loop_cm:
                _body(nc, tc, xso_d, xc_d, out_d,
                      wso_sb, wc_sb, w1_sb, w2_sb, bso_sb, bc_sb, b1_sb,
                      bits_sb, xp, fp, hp, mp, ps, thresh,
                      weight_dram=(wso_d, wc_d, w1_d))

    nc.compile()
    return nc


def _body(nc, tc, xso_d, xc_d, out_d,
          wso_sb, wc_sb, w1_sb, w2_sb, bso_sb, bc_sb, b1_sb,
          bits_sb, xp, fp, hp, mp, ps, thresh, weight_dram=None):
    from concourse.tile import add_dep_helper

    # All big HBM reads ride one ordered gpsimd stream, chained with
    # stride 4 (transfer i waits on i-4's completion): the SDMA engines
    # round-robin across everything outstanding, so without this the
    # first-needed transfer finishes no earlier than the whole burst;
    # with it the stream drains in consumption order with a few
    # transfers of lookahead.
    chain = []

    def chained_dma(out_ap, in_ap):
        h = nc.gpsimd.dma_start(out_ap, in_ap)
        if len(chain) >= 4:
            add_dep_helper(h.ins, chain[-4].ins, reason="x-stream order")
        chain.append(h)

    wso_d, wc_d, w1_d = weight_dram
    WCOL = KC1 * 512 // 2     # wso half: k-chunks 0..7 / 8..15

    def load_x(x_d, b, sub, tag):
        """DMA x[b] into a fresh SBUF tile in `sub` chunks of k-chunks,
        returning (tile, list of per-chunk slices issued on the chain)."""
        js = KC1 // sub
        x_sb = xp.tile([128, KC1 * NHW], F16, tag=tag, name=f"x_{tag}_{b}")
        xv = x_d[b].rearrange("(p j) m -> p j m", p=128)
        xt = x_sb.rearrange("p (j m) -> p j m", j=KC1)
        return x_sb, [(xt[:, js * s:js * (s + 1), :],
                       xv[:, js * s:js * (s + 1), :]) for s in range(sub)]

    def l1_branch(x_sb, w_sb, bias_sb, f_sb, br):
        """One layer-1 branch: 8 concurrent PSUM tiles, k-outer so partial
        x is usable; same lhsT feeds both m-halves back to back."""
        pts = [[ps.tile([128, MT], F32, tag="ps", name=f"pt{br}_{m}_{o}")
                for o in range(OC1)] for m in range(2)]
        for k in range(KC1):
            for o in range(OC1):
                for m in range(2):
                    nc.tensor.matmul(
                        pts[m][o][:],
                        lhsT=w_sb[:, k * 512 + o * 128:k * 512 + o * 128 + 128],
                        rhs=x_sb[:, k * NHW + m * MT:k * NHW + m * MT + MT],
                        start=(k == 0), stop=(k == KC1 - 1))
        for m in range(2):
            for o in range(OC1):
                col = (br * OC1 + o) * NHW + m * MT
                nc.scalar.activation(
                    f_sb[:, col:col + MT], pts[m][o][:],
                    mybir.ActivationFunctionType.Prelu,
                    bias=bias_sb[:, o:o + 1], scale=1.0, alpha=SLOPE)

    m_tiles = []

    def do_l2_l3(b, f_sb):
        # ---- layer 2: h = lrelu(A1 @ f + a1), fp16 out ----
        h_sb = hp.tile([128, OC2 * NHW], F16, tag="h")
        pts2 = [[ps.tile([128, MT], F32, tag="ps", name=f"pt2_{m}_{o}")
                 for o in range(OC2)] for m in range(2)]
        for k in range(KC2):
            for o in range(OC2):
                for m in range(2):
                    nc.tensor.matmul(
                        pts2[m][o][:],
                        lhsT=w1_sb[:, k * 256 + o * 128:k * 256 + o * 128 + 128],
                        rhs=f_sb[:, k * NHW + m * MT:k * NHW + m * MT + MT],
                        start=(k == 0), stop=(k == KC2 - 1))
        for m in range(2):
            for o in range(OC2):
                col = o * NHW + m * MT
                nc.scalar.activation(
                    h_sb[:, col:col + MT], pts2[m][o][:],
                    mybir.ActivationFunctionType.Prelu,
                    bias=b1_sb[:, o:o + 1], scale=1.0, alpha=SLOPE)

        # ---- layer 3a: reduce the 49 HW positions (DVE), one m-half as
        # soon as its two activations are done. The 256-channel
        # contraction is deferred past the next batch's matmuls so the
        # in-order PE never waits on this chain. ----
        m_sb = mp.tile([128, OC2 * NN], F32, tag="m", name=f"m_{b}")
        m_v = m_sb.rearrange("p (q n) -> p q n", q=OC2)
        h_v = h_sb.rearrange("p (q n x) -> p q n x", q=OC2, n=NN)
        HN = NN // 2
        for m in range(2):
            nc.vector.reduce_sum(
                m_v[:, :, m * HN:(m + 1) * HN],
                h_v[:, :, m * HN:(m + 1) * HN, :],
                axis=mybir.AxisListType.X)
        m_tiles.append(m_sb)

    def do_l3(db):
        # layer 3b: y[n] = W2 @ m (tiny fp32 matmuls) + threshold
        # (sigmoid(mean) > 0.5 <=> sum > -49*b2). Batch b's contraction
        # is emitted during a later batch's compute so only the last
        # batch's chain sits on the kernel tail.
        ps3 = ps.tile([1, NN], F32, tag="ps", name=f"ps3_{db}")
        for q in range(OC2):
            nc.tensor.matmul(
                ps3[:], lhsT=w2_sb[:, q:q + 1],
                rhs=m_tiles[db][:, q * NN:(q + 1) * NN],
                start=(q == 0), stop=(q == OC2 - 1))
        nc.vector.tensor_scalar(
            bits_sb[0:1, db * NN:(db + 1) * NN], ps3[:], float(thresh),
            None, mybir.AluOpType.is_gt)

    # ---- batches 0+1 are DMA-critical: run BOTH so-branches first, so
    # the stream prefix is wso + two x_so tensors (wc defers until the
    # PE is ~2 branches in), with weight chunks interleaved right before
    # the k-chunks that consume them. ----
    xso0_sb, so0_chunks = load_x(xso_d, 0, 8, "xso")
    xso1_sb, so1_chunks = load_x(xso_d, 1, 4, "xso")
    xc0_sb, c0_chunks = load_x(xc_d, 0, 8, "xc")
    xc1_sb, c1_chunks = load_x(xc_d, 1, 4, "xc")
    chained_dma(wso_sb[:, :WCOL], wso_d[:, :WCOL])
    for sl in so0_chunks[:4]:
        chained_dma(*sl)
    chained_dma(wso_sb[:, WCOL:], wso_d[:, WCOL:])
    for sl in so0_chunks[4:]:
        chained_dma(*sl)
    for sl in so1_chunks:
        chained_dma(*sl)
    for q in range(4):
        chained_dma(wc_sb[:, q * 2048:(q + 1) * 2048],
                    wc_d[:, q * 2048:(q + 1) * 2048])
        chained_dma(*c0_chunks[2 * q])
        if q == 3:
            chained_dma(w1_sb[:], w1_d[:])
        chained_dma(*c0_chunks[2 * q + 1])
    for sl in c1_chunks:
        chained_dma(*sl)

    f0_sb = fp.tile([128, 2 * OC1 * NHW], F16, tag="f", name="f_0")
    f1_sb = fp.tile([128, 2 * OC1 * NHW], F16, tag="f", name="f_1")
    l1_branch(xso0_sb, wso_sb, bso_sb, f0_sb, 0)
    l1_branch(xso1_sb, wso_sb, bso_sb, f1_sb, 0)
    l1_branch(xc0_sb, wc_sb, bc_sb, f0_sb, 1)
    l1_branch(xc1_sb, wc_sb, bc_sb, f1_sb, 1)
    do_l2_l3(0, f0_sb)
    do_l2_l3(1, f1_sb)
    do_l3(0)

    # ---- batches 2+3: DMA is well ahead; simple per-batch order ----
    for b in (2, 3):
        xso_sb, so_chunks = load_x(xso_d, b, 4, "xso")
        xc_sb, c_chunks = load_x(xc_d, b, 4, "xc")
        for sl in so_chunks:
            chained_dma(*sl)
        for sl in c_chunks:
            chained_dma(*sl)
        f_sb = fp.tile([128, 2 * OC1 * NHW], F16, tag="f", name=f"f_{b}")
        l1_branch(xso_sb, wso_sb, bso_sb, f_sb, 0)
        l1_branch(xc_sb, wc_sb, bc_sb, f_sb, 1)
        do_l2_l3(b, f_sb)
        do_l3(b - 1)
    do_l3(3)

    nc.sync.dma_start(out_d[:], bits_sb[0:1, :])


_CACHE = {}


def _get_nc(thresh, loop=1):
    key = (round(thresh, 9), loop)
    if key not in _CACHE:
        _CACHE[key] = build_bass(thresh, loop)
    return _CACHE[key]


def kernel(**inputs):
    wso, wc, w1, w2, bso, bc, b1, thresh = _fold_params(inputs)
    xso = np.ascontiguousarray(
        np.asarray(inputs["x_so"], dtype=np.float32).reshape(B, C, NHW))
    xc = np.ascontiguousarray(
        np.asarray(inputs["x_c"], dtype=np.float32).reshape(B, C, NHW))

    nc = _get_nc(thresh)
    in_maps = []
    for i in range(N_CORES):
        in_maps.append({
            "x_so": xso[i * BPC:(i + 1) * BPC],
            "x_c": xc[i * BPC:(i + 1) * BPC],
            "wso": wso, "wc": wc, "w1": w1, "w2": w2,
            "bso": bso, "bc": bc, "b1": b1,
        })
    res = run_bass_kernel_spmd(nc, in_maps, list(range(N_CORES)))
    out = np.concatenate([res.results[i]["out"].reshape(BPC, NN)
                          for i in range(N_CORES)], axis=0)
    return np.ascontiguousarray(out.reshape(B, NN, 1).astype(np.float32))


# revision 10
# speedup vs baseline: 1.0111x; 1.0111x over previous
"""Trainium2 Bass kernel for nn_ContextGatingSigmoidClassifier.

Math (eval mode):
  f_so = lrelu(W_so @ bn_so(x_so) + b_so)        x: [B,2048,N,H,W]
  f_c  = lrelu(W_c  @ bn_c(x_c)  + b_c)
  f    = concat -> bn1 -> W1 -> bn2 -> lrelu -> W2 -> mean(H,W) -> sigmoid > 0.5

All BatchNorms are eval-mode affine maps, so they fold into the adjacent
linear layers (done host-side in fp64):
  A_so = W_so * s_so ; a_so = W_so @ t_so + b_so          (s,t: bn scale/shift)
  A1   = diag(s2) W1 diag(s1) ; a1 = s2*(W1 @ t1 + b1) + t2
Final threshold: sigmoid(mean) > 0.5  <=>  sum_hw(W2 @ h) > -49*b2.

Device mapping: data-parallel over batch (4 per core, 8 cores). Weights
replicated. Per core, per batch element b:
  x[b] is [2048, 588] (channels x positions). Matmuls keep channels on
  SBUF partitions (K-chunks of 128), positions on the free dim (2 tiles
  of 294 = one PSUM bank each). All matmul operands fp16 (PSUM accum is
  fp32); fp32->fp16 happens inside the input DMA (SWDGE cast).
  Channel->partition mapping for x is interleaved (partition p holds
  channels 16p..16p+15) so each DMA descriptor reads contiguous runs;
  layer-1 weights are permuted host-side to match.

Schedule: one ordered gpsimd DMA chain delivers weights and x chunks in
exactly the order the PE consumes them. Layer-1 accumulates k-sub-outer
across all 8 PSUM tiles of a branch, so compute starts after the first
x sub-DMA instead of after the whole tensor; both m-halves share each
loaded weight tile back to back. Batches 0+1 run their so-branches
before any c-branch so the DMA-critical prefix is wso + two x_so
tensors (wc defers ~2 branch-times). The final layer reduces h over
the 49 HW positions first (DVE, per m-half as its activations land),
then contracts 256 channels with a tiny fp32 matmul; each batch's
final contraction is deferred past the next batch's matmuls so the
in-order PE never stalls on the activation->reduce chain, and only the
last batch's chain sits on the kernel tail.
"""

import numpy as np

import concourse.bass as bass  # noqa: F401  (engine types referenced via nc)
import concourse.tile as tile
from concourse import bacc, mybir
from concourse.bass_utils import run_bass_kernel_spmd

F16 = mybir.dt.float16
F32 = mybir.dt.float32

B, C, NN, HW = 32, 2048, 12, 49
NHW = NN * HW            # 588
N_CORES = 8
BPC = B // N_CORES       # 4 batch elements per core
MT = NHW // 2            # 294 columns = one PSUM bank of fp32
KC1 = C // 128           # 16 K-chunks, layer 1
OC1 = 512 // 128         # 4 output chunks, layer 1 (per branch)
KC2 = 1024 // 128        # 8 K-chunks, layer 2
OC2 = 256 // 128         # 2 output chunks, layer 2
EPS = 1e-5
SLOPE = 0.2


def _fold_params(d):
    """Fold BNs into linears, in fp64. Returns device-layout arrays."""
    g = {k: np.asarray(v, dtype=np.float64) for k, v in d.items()}

    def bn_st(p):
        s = g[f"{p}_g"] / np.sqrt(g[f"{p}_v"] + EPS)
        t = g[f"{p}_b"] - g[f"{p}_m"] * s
        return s, t

    s_so, t_so = bn_st("bn_so")
    s_c, t_c = bn_st("bn_c")
    s1, t1 = bn_st("bn1")
    s2, t2 = bn_st("bn2")

    A_so = g["W_so"] * s_so[None, :]                 # [512, 2048]
    a_so = g["W_so"] @ t_so + g["b_so"]              # [512]
    A_c = g["W_c"] * s_c[None, :]
    a_c = g["W_c"] @ t_c + g["b_c"]
    A1 = s2[:, None] * (g["W1"] * s1[None, :])       # [256, 1024]
    a1 = s2 * (g["W1"] @ t1 + g["b1"]) + t2          # [256]

    # layer-1 weights: chunk j holds channel 16p+j at partition p (matches
    # the contiguous-run x DMA layout). Stored p-major [128, k*m] so each
    # DMA descriptor is one contiguous per-partition run.
    def l1_prep(A):  # [512, 2048] -> [128, 16*512] fp16
        AT = A.T.reshape(128, 16, 512)               # [p, j, m] with ch = 16p+j
        return np.ascontiguousarray(AT.reshape(128, KC1 * 512)).astype(np.float16)

    wso = l1_prep(A_so)
    wc = l1_prep(A_c)
    w1 = np.ascontiguousarray(
        A1.T.reshape(KC2, 128, 256).transpose(1, 0, 2).reshape(128, KC2 * 256)
    ).astype(np.float16)
    w2 = np.ascontiguousarray(g["W2"].reshape(OC2, 128).T).astype(np.float32)  # [128, 2]
    bso = np.ascontiguousarray(a_so.reshape(OC1, 128).T).astype(np.float32)    # [128, 4]
    bc = np.ascontiguousarray(a_c.reshape(OC1, 128).T).astype(np.float32)
    b1 = np.ascontiguousarray(a1.reshape(OC2, 128).T).astype(np.float32)       # [128, 2]
    thresh = float(-HW * g["b2"][0])
    return wso, wc, w1, w2, bso, bc, b1, thresh


def build_bass(thresh, loop=1):
    """loop: on-device For_i wrapper around the whole body (timing only)."""
    nc = bacc.Bacc("TRN2", target_bir_lowering=False, debug=False)

    xso_d = nc.dram_tensor("x_so", [BPC, C, NHW], F32, kind="ExternalInput").ap()
    xc_d = nc.dram_tensor("x_c", [BPC, C, NHW], F32, kind="ExternalInput").ap()
    wso_d = nc.dram_tensor("wso", [128, KC1 * 512], F16, kind="ExternalInput").ap()
    wc_d = nc.dram_tensor("wc", [128, KC1 * 512], F16, kind="ExternalInput").ap()
    w1_d = nc.dram_tensor("w1", [128, KC2 * 256], F16, kind="ExternalInput").ap()
    w2_d = nc.dram_tensor("w2", [128, OC2], F32, kind="ExternalInput").ap()
    bso_d = nc.dram_tensor("bso", [128, OC1], F32, kind="ExternalInput").ap()
    bc_d = nc.dram_tensor("bc", [128, OC1], F32, kind="ExternalInput").ap()
    b1_d = nc.dram_tensor("b1", [128, OC2], F32, kind="ExternalInput").ap()
    out_d = nc.dram_tensor("out", [BPC * NN], F32, kind="ExternalOutput").ap()

    with tile.TileContext(nc) as tc:
        with (
            tc.tile_pool(name="wp", bufs=1) as wp,
            tc.tile_pool(name="xp", bufs=2) as xp,
            tc.tile_pool(name="fp", bufs=2) as fp,
            tc.tile_pool(name="hp", bufs=2) as hp,
            tc.tile_pool(name="mp", bufs=4) as mp,
            tc.tile_pool(name="ap", bufs=1) as ac,
            tc.tile_pool(name="ps", bufs=8, space="PSUM") as ps,
        ):
            # ---- biases / small tensors on the HWDGE (sync) ring ----
            bso_sb = wp.tile([128, OC1], F32)
            nc.sync.dma_start(bso_sb[:], bso_d[:])
            bc_sb = wp.tile([128, OC1], F32)
            nc.sync.dma_start(bc_sb[:], bc_d[:])
            b1_sb = wp.tile([128, OC2], F32)
            nc.sync.dma_start(b1_sb[:], b1_d[:])
            w2_sb = wp.tile([128, OC2], F32)
            nc.sync.dma_start(w2_sb[:], w2_d[:])
            # big weights ride the ordered gpsimd chain (see _body),
            # interleaved with the x stream at their consumption points.
            wso_sb = wp.tile([128, KC1 * 512], F16)
            wc_sb = wp.tile([128, KC1 * 512], F16)
            w1_sb = wp.tile([128, KC2 * 256], F16)

            bits_sb = ac.tile([1, BPC * NN], F32)

            import contextlib
            loop_cm = tc.For_i(0, loop, 1) if loop > 1 else contextlib.nullcontext()
            with loop_cm:
                _body(nc, tc, xso_d, xc_d, out_d,
                      wso_sb, wc_sb, w1_sb, w2_sb, bso_sb, bc_sb, b1_sb,
                      bits_sb, xp, fp, hp, mp, ps, thresh,
                      weight_dram=(wso_d, wc_d, w1_d))

    nc.compile()
    return nc


def _body(nc, tc, xso_d, xc_d, out_d,
          wso_sb, wc_sb, w1_sb, w2_sb, bso_sb, bc_sb, b1_sb,
          bits_sb, xp, fp, hp, mp, ps, thresh, weight_dram=None):
    from concourse.tile import add_dep_helper

    # All big HBM reads ride one ordered gpsimd stream, chained with
    # stride 4 (transfer i waits on i-4's completion): the SDMA engines
    # round-robin across everything outstanding, so without this the
    # first-needed transfer finishes no earlier than the whole burst;
    # with it the stream drains in consumption order with a few
    # transfers of lookahead.
    chain = []

    def chained_dma(out_ap, in_ap):
        h = nc.gpsimd.dma_start(out_ap, in_ap)
        if len(chain) >= 4:
            add_dep_helper(h.ins, chain[-4].ins, reason="x-stream order")
        chain.append(h)

    wso_d, wc_d, w1_d = weight_dram
    WCOL = KC1 * 512 // 2     # wso half: k-chunks 0..7 / 8..15

    def load_x(x_d, b, sub, tag):
        """DMA x[b] into a fresh SBUF tile, split into chunks of k-chunks
        (uniform count `sub`, or an explicit list of k-chunk counts),
        returning (tile, list of per-chunk slices issued on the chain)."""
        sizes = sub if isinstance(sub, list) else [KC1 // sub] * sub
        assert sum(sizes) == KC1
        x_sb = xp.tile([128, KC1 * NHW], F16, tag=tag, name=f"x_{tag}_{b}")
        xv = x_d[b].rearrange("(p j) m -> p j m", p=128)
        xt = x_sb.rearrange("p (j m) -> p j m", j=KC1)
        chunks, j = [], 0
        for sz in sizes:
            chunks.append((xt[:, j:j + sz, :], xv[:, j:j + sz, :]))
            j += sz
        return x_sb, chunks

    def l1_branch(x_sb, w_sb, bias_sb, f_sb, br):
        """One layer-1 branch: 8 concurrent PSUM tiles, k-outer so partial
        x is usable; same lhsT feeds both m-halves back to back."""
        pts = [[ps.tile([128, MT], F32, tag="ps", name=f"pt{br}_{m}_{o}")
                for o in range(OC1)] for m in range(2)]
        for k in range(KC1):
            for o in range(OC1):
                for m in range(2):
                    nc.tensor.matmul(
                        pts[m][o][:],
                        lhsT=w_sb[:, k * 512 + o * 128:k * 512 + o * 128 + 128],
                        rhs=x_sb[:, k * NHW + m * MT:k * NHW + m * MT + MT],
                        start=(k == 0), stop=(k == KC1 - 1))
        for m in range(2):
            for o in range(OC1):
                col = (br * OC1 + o) * NHW + m * MT
                nc.scalar.activation(
                    f_sb[:, col:col + MT], pts[m][o][:],
                    mybir.ActivationFunctionType.Prelu,
                    bias=bias_sb[:, o:o + 1], scale=1.0, alpha=SLOPE)

    m_tiles = []

    def do_l2_l3(b, f_sb):
        # ---- layer 2: h = lrelu(A1 @ f + a1), fp16 out ----
        h_sb = hp.tile([128, OC2 * NHW], F16, tag="h")
        pts2 = [[ps.tile([128, MT], F32, tag="ps", name=f"pt2_{m}_{o}")
                 for o in range(OC2)] for m in range(2)]
        for k in range(KC2):
            for o in range(OC2):
                for m in range(2):
                    nc.tensor.matmul(
                        pts2[m][o][:],
                        lhsT=w1_sb[:, k * 256 + o * 128:k * 256 + o * 128 + 128],
                        rhs=f_sb[:, k * NHW + m * MT:k * NHW + m * MT + MT],
                        start=(k == 0), stop=(k == KC2 - 1))
        for m in range(2):
            for o in range(OC2):
                col = o * NHW + m * MT
                nc.scalar.activation(
                    h_sb[:, col:col + MT], pts2[m][o][:],
                    mybir.ActivationFunctionType.Prelu,
                    bias=b1_sb[:, o:o + 1], scale=1.0, alpha=SLOPE)

        # ---- layer 3a: reduce the 49 HW positions (DVE), one m-half as
        # soon as its two activations are done. The 256-channel
        # contraction is deferred past the next batch's matmuls so the
        # in-order PE never waits on this chain. ----
        m_sb = mp.tile([128, OC2 * NN], F32, tag="m", name=f"m_{b}")
        m_v = m_sb.rearrange("p (q n) -> p q n", q=OC2)
        h_v = h_sb.rearrange("p (q n x) -> p q n x", q=OC2, n=NN)
        HN = NN // 2
        for m in range(2):
            nc.vector.reduce_sum(
                m_v[:, :, m * HN:(m + 1) * HN],
                h_v[:, :, m * HN:(m + 1) * HN, :],
                axis=mybir.AxisListType.X)
        m_tiles.append(m_sb)

    def do_l3(db):
        # layer 3b: y[n] = W2 @ m (tiny fp32 matmuls) + threshold
        # (sigmoid(mean) > 0.5 <=> sum > -49*b2). Batch b's contraction
        # is emitted during a later batch's compute so only the last
        # batch's chain sits on the kernel tail.
        ps3 = ps.tile([1, NN], F32, tag="ps", name=f"ps3_{db}")
        for q in range(OC2):
            nc.tensor.matmul(
                ps3[:], lhsT=w2_sb[:, q:q + 1],
                rhs=m_tiles[db][:, q * NN:(q + 1) * NN],
                start=(q == 0), stop=(q == OC2 - 1))
        nc.vector.tensor_scalar(
            bits_sb[0:1, db * NN:(db + 1) * NN], ps3[:], float(thresh),
            None, mybir.AluOpType.is_gt)

    # ---- batches 0+1 are DMA-critical: run BOTH so-branches first, so
    # the stream prefix is wso + two x_so tensors (wc defers until the
    # PE is ~2 branches in), with weight chunks interleaved right before
    # the k-chunks that consume them. ----
    # finest granules first so the PE's first matmul waits only ~1/4 of
    # wso + one k-chunk of x_so; weight quarter q covers k-chunks 4q..4q+3
    xso0_sb, so0_chunks = load_x(xso_d, 0, [1, 1, 1, 1, 2, 2, 2, 2, 2, 2], "xso")
    xso1_sb, so1_chunks = load_x(xso_d, 1, 4, "xso")
    xc0_sb, c0_chunks = load_x(xc_d, 0, 8, "xc")
    xc1_sb, c1_chunks = load_x(xc_d, 1, 4, "xc")
    WQ = KC1 * 512 // 4
    chained_dma(wso_sb[:, :WQ], wso_d[:, :WQ])           # k0-3
    for sl in so0_chunks[:4]:                            # k0..k3 singly
        chained_dma(*sl)
    chained_dma(wso_sb[:, WQ:2 * WQ], wso_d[:, WQ:2 * WQ])   # k4-7
    for sl in so0_chunks[4:6]:                           # k4,5 / k6,7
        chained_dma(*sl)
    chained_dma(wso_sb[:, 2 * WQ:3 * WQ], wso_d[:, 2 * WQ:3 * WQ])
    for sl in so0_chunks[6:8]:                           # k8,9 / k10,11
        chained_dma(*sl)
    chained_dma(wso_sb[:, 3 * WQ:], wso_d[:, 3 * WQ:])
    for sl in so0_chunks[8:]:                            # k12,13 / k14,15
        chained_dma(*sl)
    for sl in so1_chunks:
        chained_dma(*sl)
    for q in range(4):
        chained_dma(wc_sb[:, q * 2048:(q + 1) * 2048],
                    wc_d[:, q * 2048:(q + 1) * 2048])
        chained_dma(*c0_chunks[2 * q])
        if q == 3:
            chained_dma(w1_sb[:], w1_d[:])
        chained_dma(*c0_chunks[2 * q + 1])
    for sl in c1_chunks:
        chained_dma(*sl)

    f0_sb = fp.tile([128, 2 * OC1 * NHW], F16, tag="f", name="f_0")
    f1_sb = fp.tile([128, 2 * OC1 * NHW], F16, tag="f", name="f_1")
    l1_branch(xso0_sb, wso_sb, bso_sb, f0_sb, 0)
    l1_branch(xso1_sb, wso_sb, bso_sb, f1_sb, 0)
    l1_branch(xc0_sb, wc_sb, bc_sb, f0_sb, 1)
    l1_branch(xc1_sb, wc_sb, bc_sb, f1_sb, 1)
    do_l2_l3(0, f0_sb)
    do_l2_l3(1, f1_sb)
    do_l3(0)

    # ---- batches 2+3: DMA is well ahead; simple per-batch order ----
    for b in (2, 3):
        xso_sb, so_chunks = load_x(xso_d, b, 4, "xso")
        xc_sb, c_chunks = load_x(xc_d, b, 4, "xc")
        for sl in so_chunks:
            chained_dma(*sl)
        for sl in c_chunks:
            chained_dma(*sl)
        f_sb = fp.tile([128, 2 * OC1 * NHW], F16, tag="f", name=f"f_{b}")
        l1_branch(xso_sb, wso_sb, bso_sb, f_sb, 0)
        l1_branch(xc_sb, wc_sb, bc_sb, f_sb, 1)
        do_l2_l3(b, f_sb)
        do_l3(b - 1)
    do_l3(3)

    nc.sync.dma_start(out_d[:], bits_sb[0:1, :])


_CACHE = {}


def _get_nc(thresh, loop=1):
    key = (round(thresh, 9), loop)
    if key not in _CACHE:
        _CACHE[key] = build_bass(thresh, loop)
    return _CACHE[key]


def kernel(**inputs):
    wso, wc, w1, w2, bso, bc, b1, thresh = _fold_params(inputs)
    xso = np.ascontiguousarray(
        np.asarray(inputs["x_so"], dtype=np.float32).reshape(B, C, NHW))
    xc = np.ascontiguousarray(
        np.asarray(inputs["x_c"], dtype=np.float32).reshape(B, C, NHW))

    nc = _get_nc(thresh)
    in_maps = []
    for i in range(N_CORES):
        in_maps.append({
            "x_so": xso[i * BPC:(i + 1) * BPC],
            "x_c": xc[i * BPC:(i + 1) * BPC],
            "wso": wso, "wc": wc, "w1": w1, "w2": w2,
            "bso": bso, "bc": bc, "b1": b1,
        })
    res = run_bass_kernel_spmd(nc, in_maps, list(range(N_CORES)))
    out = np.concatenate([res.results[i]["out"].reshape(BPC, NN)
                          for i in range(N_CORES)], axis=0)
    return np.ascontiguousarray(out.reshape(B, NN, 1).astype(np.float32))


# revision 13
# speedup vs baseline: 1.0197x; 1.0085x over previous
"""Trainium2 Bass kernel for nn_ContextGatingSigmoidClassifier.

Math (eval mode):
  f_so = lrelu(W_so @ bn_so(x_so) + b_so)        x: [B,2048,N,H,W]
  f_c  = lrelu(W_c  @ bn_c(x_c)  + b_c)
  f    = concat -> bn1 -> W1 -> bn2 -> lrelu -> W2 -> mean(H,W) -> sigmoid > 0.5

All BatchNorms are eval-mode affine maps, so they fold into the adjacent
linear layers (done host-side in fp64):
  A_so = W_so * s_so ; a_so = W_so @ t_so + b_so          (s,t: bn scale/shift)
  A1   = diag(s2) W1 diag(s1) ; a1 = s2*(W1 @ t1 + b1) + t2
Final threshold: sigmoid(mean) > 0.5  <=>  sum_hw(W2 @ h) > -49*b2.

Device mapping: data-parallel over batch (4 per core, 8 cores). Weights
replicated. Per core, per batch element b:
  x[b] is [2048, 588] (channels x positions). Matmuls keep channels on
  SBUF partitions (K-chunks of 128), positions on the free dim (2 tiles
  of 294 = one PSUM bank each). All matmul operands fp16 (PSUM accum is
  fp32); fp32->fp16 happens inside the input DMA (SWDGE cast).
  Channel->partition mapping for x is interleaved (partition p holds
  channels 16p..16p+15) so each DMA descriptor reads contiguous runs;
  layer-1 weights are permuted host-side to match.

Schedule: one ordered gpsimd DMA chain delivers weights and x chunks in
exactly the order the PE consumes them. Layer-1 accumulates k-sub-outer
across all 8 PSUM tiles of a branch, so compute starts after the first
x sub-DMA instead of after the whole tensor; both m-halves share each
loaded weight tile back to back. Batches 0+1 run their so-branches
before any c-branch so the DMA-critical prefix is wso + two x_so
tensors (wc defers ~2 branch-times). The final layer reduces h over
the 49 HW positions first (DVE, per m-half as its activations land),
then contracts 256 channels with a tiny fp32 matmul; each batch's
final contraction is deferred past the next batch's matmuls so the
in-order PE never stalls on the activation->reduce chain, and only the
last batch's chain sits on the kernel tail.
"""

import numpy as np

import concourse.bass as bass  # noqa: F401  (engine types referenced via nc)
import concourse.tile as tile
from concourse import bacc, mybir
from concourse.bass_utils import run_bass_kernel_spmd

F16 = mybir.dt.float16
F32 = mybir.dt.float32

B, C, NN, HW = 32, 2048, 12, 49
NHW = NN * HW            # 588
N_CORES = 8
BPC = B // N_CORES       # 4 batch elements per core
MT = NHW // 2            # 294 columns = one PSUM bank of fp32
KC1 = C // 128           # 16 K-chunks, layer 1
OC1 = 512 // 128         # 4 output chunks, layer 1 (per branch)
KC2 = 1024 // 128        # 8 K-chunks, layer 2
OC2 = 256 // 128         # 2 output chunks, layer 2
EPS = 1e-5
SLOPE = 0.2


def _fold_params(d):
    """Fold BNs into linears, in fp64. Returns device-layout arrays."""
    g = {k: np.asarray(v, dtype=np.float64) for k, v in d.items()}

    def bn_st(p):
        s = g[f"{p}_g"] / np.sqrt(g[f"{p}_v"] + EPS)
        t = g[f"{p}_b"] - g[f"{p}_m"] * s
        return s, t

    s_so, t_so = bn_st("bn_so")
    s_c, t_c = bn_st("bn_c")
    s1, t1 = bn_st("bn1")
    s2, t2 = bn_st("bn2")

    A_so = g["W_so"] * s_so[None, :]                 # [512, 2048]
    a_so = g["W_so"] @ t_so + g["b_so"]              # [512]
    A_c = g["W_c"] * s_c[None, :]
    a_c = g["W_c"] @ t_c + g["b_c"]
    A1 = s2[:, None] * (g["W1"] * s1[None, :])       # [256, 1024]
    a1 = s2 * (g["W1"] @ t1 + g["b1"]) + t2          # [256]

    # layer-1 weights: chunk j holds channel 16p+j at partition p (matches
    # the contiguous-run x DMA layout). Stored p-major [128, k*m] so each
    # DMA descriptor is one contiguous per-partition run.
    def l1_prep(A):  # [512, 2048] -> [128, 16*512] fp16
        AT = A.T.reshape(128, 16, 512)               # [p, j, m] with ch = 16p+j
        return np.ascontiguousarray(AT.reshape(128, KC1 * 512)).astype(np.float16)

    wso = l1_prep(A_so)
    wc = l1_prep(A_c)
    w1 = np.ascontiguousarray(
        A1.T.reshape(KC2, 128, 256).transpose(1, 0, 2).reshape(128, KC2 * 256)
    ).astype(np.float16)
    w2 = np.ascontiguousarray(g["W2"].reshape(OC2, 128).T).astype(np.float32)  # [128, 2]
    bso = np.ascontiguousarray(a_so.reshape(OC1, 128).T).astype(np.float32)    # [128, 4]
    bc = np.ascontiguousarray(a_c.reshape(OC1, 128).T).astype(np.float32)
    b1 = np.ascontiguousarray(a1.reshape(OC2, 128).T).astype(np.float32)       # [128, 2]
    thresh = float(-HW * g["b2"][0])
    return wso, wc, w1, w2, bso, bc, b1, thresh


def build_bass(thresh, loop=1):
    """loop: on-device For_i wrapper around the whole body (timing only)."""
    nc = bacc.Bacc("TRN2", target_bir_lowering=False, debug=False)

    xso_d = nc.dram_tensor("x_so", [BPC, C, NHW], F32, kind="ExternalInput").ap()
    xc_d = nc.dram_tensor("x_c", [BPC, C, NHW], F32, kind="ExternalInput").ap()
    wso_d = nc.dram_tensor("wso", [128, KC1 * 512], F16, kind="ExternalInput").ap()
    wc_d = nc.dram_tensor("wc", [128, KC1 * 512], F16, kind="ExternalInput").ap()
    w1_d = nc.dram_tensor("w1", [128, KC2 * 256], F16, kind="ExternalInput").ap()
    w2_d = nc.dram_tensor("w2", [128, OC2], F32, kind="ExternalInput").ap()
    bso_d = nc.dram_tensor("bso", [128, OC1], F32, kind="ExternalInput").ap()
    bc_d = nc.dram_tensor("bc", [128, OC1], F32, kind="ExternalInput").ap()
    b1_d = nc.dram_tensor("b1", [128, OC2], F32, kind="ExternalInput").ap()
    out_d = nc.dram_tensor("out", [BPC * NN], F32, kind="ExternalOutput").ap()

    with tile.TileContext(nc) as tc:
        with (
            tc.tile_pool(name="wp", bufs=1) as wp,
            tc.tile_pool(name="xp", bufs=2) as xp,
            tc.tile_pool(name="fp", bufs=2) as fp,
            tc.tile_pool(name="hp", bufs=2) as hp,
            tc.tile_pool(name="mp", bufs=4) as mp,
            tc.tile_pool(name="ap", bufs=1) as ac,
            tc.tile_pool(name="ps", bufs=8, space="PSUM") as ps,
        ):
            # ---- biases / small tensors on the HWDGE (sync) ring ----
            bso_sb = wp.tile([128, OC1], F32)
            nc.sync.dma_start(bso_sb[:], bso_d[:])
            bc_sb = wp.tile([128, OC1], F32)
            nc.sync.dma_start(bc_sb[:], bc_d[:])
            b1_sb = wp.tile([128, OC2], F32)
            nc.sync.dma_start(b1_sb[:], b1_d[:])
            w2_sb = wp.tile([128, OC2], F32)
            nc.sync.dma_start(w2_sb[:], w2_d[:])
            # big weights ride the ordered gpsimd chain (see _body),
            # interleaved with the x stream at their consumption points.
            wso_sb = wp.tile([128, KC1 * 512], F16)
            wc_sb = wp.tile([128, KC1 * 512], F16)
            w1_sb = wp.tile([128, KC2 * 256], F16)

            bits_sb = ac.tile([1, BPC * NN], F32)

            import contextlib
            loop_cm = tc.For_i(0, loop, 1) if loop > 1 else contextlib.nullcontext()
            with loop_cm:
                _body(nc, tc, xso_d, xc_d, out_d,
                      wso_sb, wc_sb, w1_sb, w2_sb, bso_sb, bc_sb, b1_sb,
                      bits_sb, xp, fp, hp, mp, ps, thresh,
                      weight_dram=(wso_d, wc_d, w1_d))

    nc.compile()
    return nc


def _body(nc, tc, xso_d, xc_d, out_d,
          wso_sb, wc_sb, w1_sb, w2_sb, bso_sb, bc_sb, b1_sb,
          bits_sb, xp, fp, hp, mp, ps, thresh, weight_dram=None):
    from concourse.tile import add_dep_helper

    # All big HBM reads ride one ordered gpsimd stream, chained with
    # stride 4 (transfer i waits on i-4's completion): the SDMA engines
    # round-robin across everything outstanding, so without this the
    # first-needed transfer finishes no earlier than the whole burst;
    # with it the stream drains in consumption order with a few
    # transfers of lookahead.
    chain = []

    def chained_dma(out_ap, in_ap):
        h = nc.gpsimd.dma_start(out_ap, in_ap)
        if len(chain) >= 4:
            add_dep_helper(h.ins, chain[-4].ins, reason="x-stream order")
        chain.append(h)

    wso_d, wc_d, w1_d = weight_dram
    WCOL = KC1 * 512 // 2     # wso half: k-chunks 0..7 / 8..15

    def load_x(x_d, b, sub, tag):
        """DMA x[b] into a fresh SBUF tile, split into chunks of k-chunks
        (uniform count `sub`, or an explicit list of k-chunk counts),
        returning (tile, list of per-chunk slices issued on the chain)."""
        sizes = sub if isinstance(sub, list) else [KC1 // sub] * sub
        assert sum(sizes) == KC1
        x_sb = xp.tile([128, KC1 * NHW], F16, tag=tag, name=f"x_{tag}_{b}")
        xv = x_d[b].rearrange("(p j) m -> p j m", p=128)
        xt = x_sb.rearrange("p (j m) -> p j m", j=KC1)
        chunks, j = [], 0
        for sz in sizes:
            chunks.append((xt[:, j:j + sz, :], xv[:, j:j + sz, :]))
            j += sz
        return x_sb, chunks

    def l1_branch(x_sb, w_sb, bias_sb, f_sb, br):
        """One layer-1 branch: 8 concurrent PSUM tiles, k-outer so partial
        x is usable; same lhsT feeds both m-halves back to back."""
        pts = [[ps.tile([128, MT], F32, tag="ps", name=f"pt{br}_{m}_{o}")
                for o in range(OC1)] for m in range(2)]
        for k in range(KC1):
            for o in range(OC1):
                for m in range(2):
                    nc.tensor.matmul(
                        pts[m][o][:],
                        lhsT=w_sb[:, k * 512 + o * 128:k * 512 + o * 128 + 128],
                        rhs=x_sb[:, k * NHW + m * MT:k * NHW + m * MT + MT],
                        start=(k == 0), stop=(k == KC1 - 1))
        for m in range(2):
            for o in range(OC1):
                col = (br * OC1 + o) * NHW + m * MT
                nc.scalar.activation(
                    f_sb[:, col:col + MT], pts[m][o][:],
                    mybir.ActivationFunctionType.Prelu,
                    bias=bias_sb[:, o:o + 1], scale=1.0, alpha=SLOPE)

    m_tiles = []

    def do_l2_l3(b, f_sb, last=False):
        # ---- layer 2: h = lrelu(A1 @ f + a1), fp16 out ----
        # Normal batches: one k-pass over all 4 PSUM tiles (same lhsT
        # feeds both m-halves back to back). Last batch: one m-half per
        # k-pass, so its act -> reduce chain drains while the PE still
        # works the other half and only half the chain sits on the tail.
        h_sb = hp.tile([128, OC2 * NHW], F16, tag="h")
        m_sb = mp.tile([128, OC2 * NN], F32, tag="m", name=f"m_{b}")
        m_v = m_sb.rearrange("p (q n) -> p q n", q=OC2)
        h_v = h_sb.rearrange("p (q n x) -> p q n x", q=OC2, n=NN)
        HN = NN // 2

        def reduce_part(m, qs):
            # layer 3a: reduce the 49 HW positions on the DVE; runs as
            # soon as the covered activations are done
            nc.vector.reduce_sum(
                m_v[:, qs, m * HN:(m + 1) * HN],
                h_v[:, qs, m * HN:(m + 1) * HN, :],
                axis=mybir.AxisListType.X)

        def act_tile(m, o, pt):
            col = o * NHW + m * MT
            nc.scalar.activation(
                h_sb[:, col:col + MT], pt[:],
                mybir.ActivationFunctionType.Prelu,
                bias=b1_sb[:, o:o + 1], scale=1.0, alpha=SLOPE)

        def kpass(tiles_mo):
            pts2 = [ps.tile([128, MT], F32, tag="ps", name=f"pt2_{m}_{o}")
                    for m, o in tiles_mo]
            for k in range(KC2):
                for i, (m, o) in enumerate(tiles_mo):
                    nc.tensor.matmul(
                        pts2[i][:],
                        lhsT=w1_sb[:, k * 256 + o * 128:k * 256 + o * 128 + 128],
                        rhs=f_sb[:, k * NHW + m * MT:k * NHW + m * MT + MT],
                        start=(k == 0), stop=(k == KC2 - 1))
            return pts2

        if not last:
            pts2 = kpass([(m, o) for o in range(OC2) for m in range(2)])
            for i, (m, o) in enumerate([(m, o) for o in range(OC2)
                                        for m in range(2)]):
                act_tile(m, o, pts2[i])
            for m in range(2):
                reduce_part(m, slice(0, OC2))
        else:
            # last batch: one (m,o) tile per k-pass so each act + per-q
            # reduce drains under the next tile's matmuls; the kernel
            # tail carries only the final tile's act + 1-chunk reduce
            for m, o in ((1, 0), (1, 1), (0, 0), (0, 1)):
                pt = kpass([(m, o)])[0]
                act_tile(m, o, pt)
                reduce_part(m, slice(o, o + 1))
        m_tiles.append(m_sb)

    def do_l3(db):
        # layer 3b: y[n] = W2 @ m (tiny fp32 matmuls) + threshold
        # (sigmoid(mean) > 0.5 <=> sum > -49*b2). Batch b's contraction
        # is emitted during a later batch's compute so only the last
        # batch's chain sits on the kernel tail.
        ps3 = ps.tile([1, NN], F32, tag="ps", name=f"ps3_{db}")
        for q in range(OC2):
            nc.tensor.matmul(
                ps3[:], lhsT=w2_sb[:, q:q + 1],
                rhs=m_tiles[db][:, q * NN:(q + 1) * NN],
                start=(q == 0), stop=(q == OC2 - 1))
        nc.vector.tensor_scalar(
            bits_sb[0:1, db * NN:(db + 1) * NN], ps3[:], float(thresh),
            None, mybir.AluOpType.is_gt)

    # ---- batches 0+1 are DMA-critical: run BOTH so-branches first, so
    # the stream prefix is wso + two x_so tensors (wc defers until the
    # PE is ~2 branches in), with weight chunks interleaved right before
    # the k-chunks that consume them. ----
    # finest granules first so the PE's first matmul waits only ~1/4 of
    # wso + one k-chunk of x_so; weight quarter q covers k-chunks 4q..4q+3
    xso0_sb, so0_chunks = load_x(xso_d, 0, [1, 1, 1, 1, 2, 2, 2, 2, 2, 2], "xso")
    xso1_sb, so1_chunks = load_x(xso_d, 1, 4, "xso")
    xc0_sb, c0_chunks = load_x(xc_d, 0, 8, "xc")
    xc1_sb, c1_chunks = load_x(xc_d, 1, 4, "xc")
    WQ = KC1 * 512 // 4
    chained_dma(wso_sb[:, :WQ], wso_d[:, :WQ])           # k0-3
    for sl in so0_chunks[:4]:                            # k0..k3 singly
        chained_dma(*sl)
    chained_dma(wso_sb[:, WQ:2 * WQ], wso_d[:, WQ:2 * WQ])   # k4-7
    for sl in so0_chunks[4:6]:                           # k4,5 / k6,7
        chained_dma(*sl)
    chained_dma(wso_sb[:, 2 * WQ:3 * WQ], wso_d[:, 2 * WQ:3 * WQ])
    for sl in so0_chunks[6:8]:                           # k8,9 / k10,11
        chained_dma(*sl)
    chained_dma(wso_sb[:, 3 * WQ:], wso_d[:, 3 * WQ:])
    for sl in so0_chunks[8:]:                            # k12,13 / k14,15
        chained_dma(*sl)
    for sl in so1_chunks:
        chained_dma(*sl)
    for q in range(4):
        chained_dma(wc_sb[:, q * 2048:(q + 1) * 2048],
                    wc_d[:, q * 2048:(q + 1) * 2048])
        chained_dma(*c0_chunks[2 * q])
        if q == 3:
            chained_dma(w1_sb[:], w1_d[:])
        chained_dma(*c0_chunks[2 * q + 1])
    for sl in c1_chunks:
        chained_dma(*sl)

    f0_sb = fp.tile([128, 2 * OC1 * NHW], F16, tag="f", name="f_0")
    f1_sb = fp.tile([128, 2 * OC1 * NHW], F16, tag="f", name="f_1")
    l1_branch(xso0_sb, wso_sb, bso_sb, f0_sb, 0)
    l1_branch(xso1_sb, wso_sb, bso_sb, f1_sb, 0)
    l1_branch(xc0_sb, wc_sb, bc_sb, f0_sb, 1)
    l1_branch(xc1_sb, wc_sb, bc_sb, f1_sb, 1)
    do_l2_l3(0, f0_sb)
    do_l2_l3(1, f1_sb)
    do_l3(0)

    # ---- batches 2+3: DMA is well ahead; simple per-batch order ----
    for b in (2, 3):
        xso_sb, so_chunks = load_x(xso_d, b, 4, "xso")
        xc_sb, c_chunks = load_x(xc_d, b, 4, "xc")
        for sl in so_chunks:
            chained_dma(*sl)
        for sl in c_chunks:
            chained_dma(*sl)
        f_sb = fp.tile([128, 2 * OC1 * NHW], F16, tag="f", name=f"f_{b}")
        l1_branch(xso_sb, wso_sb, bso_sb, f_sb, 0)
        l1_branch(xc_sb, wc_sb, bc_sb, f_sb, 1)
        do_l2_l3(b, f_sb, last=(b == 3))
        do_l3(b - 1)
    do_l3(3)

    nc.sync.dma_start(out_d[:], bits_sb[0:1, :])


_CACHE = {}


def _get_nc(thresh, loop=1):
    key = (round(thresh, 9), loop)
    if key not in _CACHE:
        _CACHE[key] = build_bass(thresh, loop)
    return _CACHE[key]


def kernel(**inputs):
    wso, wc, w1, w2, bso, bc, b1, thresh = _fold_params(inputs)
    xso = np.ascontiguousarray(
        np.asarray(inputs["x_so"], dtype=np.float32).reshape(B, C, NHW))
    xc = np.ascontiguousarray(
        np.asarray(inputs["x_c"], dtype=np.float32).reshape(B, C, NHW))

    nc = _get_nc(thresh)
    in_maps = []
    for i in range(N_CORES):
        in_maps.append({
            "x_so": xso[i * BPC:(i + 1) * BPC],
            "x_c": xc[i * BPC:(i + 1) * BPC],
            "wso": wso, "wc": wc, "w1": w1, "w2": w2,
            "bso": bso, "bc": bc, "b1": b1,
        })
    res = run_bass_kernel_spmd(nc, in_maps, list(range(N_CORES)))
    out = np.concatenate([res.results[i]["out"].reshape(BPC, NN)
                          for i in range(N_CORES)], axis=0)
    return np.ascontiguousarray(out.reshape(B, NN, 1).astype(np.float32))


# revision 15
# speedup vs baseline: 1.0243x; 1.0045x over previous
"""Trainium2 Bass kernel for nn_ContextGatingSigmoidClassifier.

Math (eval mode):
  f_so = lrelu(W_so @ bn_so(x_so) + b_so)        x: [B,2048,N,H,W]
  f_c  = lrelu(W_c  @ bn_c(x_c)  + b_c)
  f    = concat -> bn1 -> W1 -> bn2 -> lrelu -> W2 -> mean(H,W) -> sigmoid > 0.5

All BatchNorms are eval-mode affine maps, so they fold into the adjacent
linear layers (done host-side in fp64):
  A_so = W_so * s_so ; a_so = W_so @ t_so + b_so          (s,t: bn scale/shift)
  A1   = diag(s2) W1 diag(s1) ; a1 = s2*(W1 @ t1 + b1) + t2
Final threshold: sigmoid(mean) > 0.5  <=>  sum_hw(W2 @ h) > -49*b2.

Device mapping: data-parallel over batch (4 per core, 8 cores). Weights
replicated. Per core, per batch element b:
  x[b] is [2048, 588] (channels x positions). Matmuls keep channels on
  SBUF partitions (K-chunks of 128), positions on the free dim (2 tiles
  of 294 = one PSUM bank each). All matmul operands fp16 (PSUM accum is
  fp32); fp32->fp16 happens inside the input DMA (SWDGE cast).
  Channel->partition mapping for x is interleaved (partition p holds
  channels 16p..16p+15) so each DMA descriptor reads contiguous runs;
  layer-1 weights are permuted host-side to match.

Schedule: one ordered gpsimd DMA chain delivers weights and x chunks in
exactly the order the PE consumes them. Layer-1 accumulates k-sub-outer
across all 8 PSUM tiles of a branch, so compute starts after the first
x sub-DMA instead of after the whole tensor; both m-halves share each
loaded weight tile back to back. Batches 0+1 run their so-branches
before any c-branch so the DMA-critical prefix is wso + two x_so
tensors (wc defers ~2 branch-times). The final layer reduces h over
the 49 HW positions first (DVE, per m-half as its activations land),
then contracts 256 channels with a tiny fp32 matmul; each batch's
final contraction is deferred past the next batch's matmuls so the
in-order PE never stalls on the activation->reduce chain, and only the
last batch's chain sits on the kernel tail.
"""

import numpy as np

import concourse.bass as bass  # noqa: F401  (engine types referenced via nc)
import concourse.tile as tile
from concourse import bacc, mybir
from concourse.bass_utils import run_bass_kernel_spmd

F16 = mybir.dt.float16
F32 = mybir.dt.float32

B, C, NN, HW = 32, 2048, 12, 49
NHW = NN * HW            # 588
N_CORES = 8
BPC = B // N_CORES       # 4 batch elements per core
MT = NHW // 2            # 294 columns = one PSUM bank of fp32
KC1 = C // 128           # 16 K-chunks, layer 1
OC1 = 512 // 128         # 4 output chunks, layer 1 (per branch)
KC2 = 1024 // 128        # 8 K-chunks, layer 2
OC2 = 256 // 128         # 2 output chunks, layer 2
EPS = 1e-5
SLOPE = 0.2


def _fold_params(d):
    """Fold BNs into linears, in fp64. Returns device-layout arrays."""
    g = {k: np.asarray(v, dtype=np.float64) for k, v in d.items()}

    def bn_st(p):
        s = g[f"{p}_g"] / np.sqrt(g[f"{p}_v"] + EPS)
        t = g[f"{p}_b"] - g[f"{p}_m"] * s
        return s, t

    s_so, t_so = bn_st("bn_so")
    s_c, t_c = bn_st("bn_c")
    s1, t1 = bn_st("bn1")
    s2, t2 = bn_st("bn2")

    A_so = g["W_so"] * s_so[None, :]                 # [512, 2048]
    a_so = g["W_so"] @ t_so + g["b_so"]              # [512]
    A_c = g["W_c"] * s_c[None, :]
    a_c = g["W_c"] @ t_c + g["b_c"]
    A1 = s2[:, None] * (g["W1"] * s1[None, :])       # [256, 1024]
    a1 = s2 * (g["W1"] @ t1 + g["b1"]) + t2          # [256]

    # layer-1 weights: chunk j holds channel 16p+j at partition p (matches
    # the contiguous-run x DMA layout). Stored p-major [128, k*m] so each
    # DMA descriptor is one contiguous per-partition run.
    def l1_prep(A):  # [512, 2048] -> [128, 16*512] fp16
        AT = A.T.reshape(128, 16, 512)               # [p, j, m] with ch = 16p+j
        return np.ascontiguousarray(AT.reshape(128, KC1 * 512)).astype(np.float16)

    wso = l1_prep(A_so)
    wc = l1_prep(A_c)
    w1 = np.ascontiguousarray(
        A1.T.reshape(KC2, 128, 256).transpose(1, 0, 2).reshape(128, KC2 * 256)
    ).astype(np.float16)
    w2 = np.ascontiguousarray(g["W2"].reshape(OC2, 128).T).astype(np.float32)  # [128, 2]
    bso = np.ascontiguousarray(a_so.reshape(OC1, 128).T).astype(np.float32)    # [128, 4]
    bc = np.ascontiguousarray(a_c.reshape(OC1, 128).T).astype(np.float32)
    b1 = np.ascontiguousarray(a1.reshape(OC2, 128).T).astype(np.float32)       # [128, 2]
    thresh = float(-HW * g["b2"][0])
    return wso, wc, w1, w2, bso, bc, b1, thresh


def build_bass(thresh, loop=1):
    """loop: on-device For_i wrapper around the whole body (timing only)."""
    nc = bacc.Bacc("TRN2", target_bir_lowering=False, debug=False)

    xso_d = nc.dram_tensor("x_so", [BPC, C, NHW], F32, kind="ExternalInput").ap()
    xc_d = nc.dram_tensor("x_c", [BPC, C, NHW], F32, kind="ExternalInput").ap()
    wso_d = nc.dram_tensor("wso", [128, KC1 * 512], F16, kind="ExternalInput").ap()
    wc_d = nc.dram_tensor("wc", [128, KC1 * 512], F16, kind="ExternalInput").ap()
    w1_d = nc.dram_tensor("w1", [128, KC2 * 256], F16, kind="ExternalInput").ap()
    w2_d = nc.dram_tensor("w2", [128, OC2], F32, kind="ExternalInput").ap()
    bso_d = nc.dram_tensor("bso", [128, OC1], F32, kind="ExternalInput").ap()
    bc_d = nc.dram_tensor("bc", [128, OC1], F32, kind="ExternalInput").ap()
    b1_d = nc.dram_tensor("b1", [128, OC2], F32, kind="ExternalInput").ap()
    out_d = nc.dram_tensor("out", [BPC * NN], F32, kind="ExternalOutput").ap()

    with tile.TileContext(nc) as tc:
        with (
            tc.tile_pool(name="wp", bufs=1) as wp,
            tc.tile_pool(name="xp", bufs=2) as xp,
            tc.tile_pool(name="fp", bufs=2) as fp,
            tc.tile_pool(name="hp", bufs=2) as hp,
            tc.tile_pool(name="mp", bufs=4) as mp,
            tc.tile_pool(name="ap", bufs=1) as ac,
            tc.tile_pool(name="ps", bufs=8, space="PSUM") as ps,
        ):
            # ---- biases / small tensors on the HWDGE (sync) ring ----
            bso_sb = wp.tile([128, OC1], F32)
            nc.sync.dma_start(bso_sb[:], bso_d[:])
            bc_sb = wp.tile([128, OC1], F32)
            nc.sync.dma_start(bc_sb[:], bc_d[:])
            b1_sb = wp.tile([128, OC2], F32)
            nc.sync.dma_start(b1_sb[:], b1_d[:])
            w2_sb = wp.tile([128, OC2], F32)
            nc.sync.dma_start(w2_sb[:], w2_d[:])
            # big weights ride the ordered gpsimd chain (see _body),
            # interleaved with the x stream at their consumption points.
            wso_sb = wp.tile([128, KC1 * 512], F16)
            wc_sb = wp.tile([128, KC1 * 512], F16)
            w1_sb = wp.tile([128, KC2 * 256], F16)

            bits_sb = ac.tile([1, BPC * NN], F32)

            import contextlib
            loop_cm = tc.For_i(0, loop, 1) if loop > 1 else contextlib.nullcontext()
            with loop_cm:
                _body(nc, tc, xso_d, xc_d, out_d,
                      wso_sb, wc_sb, w1_sb, w2_sb, bso_sb, bc_sb, b1_sb,
                      bits_sb, xp, fp, hp, mp, ps, thresh,
                      weight_dram=(wso_d, wc_d, w1_d))

    nc.compile()
    return nc


def _body(nc, tc, xso_d, xc_d, out_d,
          wso_sb, wc_sb, w1_sb, w2_sb, bso_sb, bc_sb, b1_sb,
          bits_sb, xp, fp, hp, mp, ps, thresh, weight_dram=None):
    from concourse.tile import add_dep_helper

    # All big HBM reads ride one ordered gpsimd stream, chained with
    # stride 4 (transfer i waits on i-4's completion): the SDMA engines
    # round-robin across everything outstanding, so without this the
    # first-needed transfer finishes no earlier than the whole burst;
    # with it the stream drains in consumption order with a few
    # transfers of lookahead.
    chain = []

    def chained_dma(out_ap, in_ap):
        h = nc.gpsimd.dma_start(out_ap, in_ap)
        if len(chain) >= 4:
            add_dep_helper(h.ins, chain[-4].ins, reason="x-stream order")
        chain.append(h)

    wso_d, wc_d, w1_d = weight_dram
    WCOL = KC1 * 512 // 2     # wso half: k-chunks 0..7 / 8..15

    def load_x(x_d, b, sub, tag):
        """DMA x[b] into a fresh SBUF tile, split into chunks of k-chunks
        (uniform count `sub`, or an explicit list of k-chunk counts),
        returning (tile, list of per-chunk slices issued on the chain)."""
        sizes = sub if isinstance(sub, list) else [KC1 // sub] * sub
        assert sum(sizes) == KC1
        x_sb = xp.tile([128, KC1 * NHW], F16, tag=tag, name=f"x_{tag}_{b}")
        xv = x_d[b].rearrange("(p j) m -> p j m", p=128)
        xt = x_sb.rearrange("p (j m) -> p j m", j=KC1)
        chunks, j = [], 0
        for sz in sizes:
            chunks.append((xt[:, j:j + sz, :], xv[:, j:j + sz, :]))
            j += sz
        return x_sb, chunks

    def l1_branch(x_sb, w_sb, bias_sb, f_sb, br, korder="sub"):
        """One layer-1 branch. korder="sub": 8 concurrent PSUM tiles,
        k-outer, so partial x is usable while the tensor still streams
        in (required wherever the DMA isn't ahead); same lhsT feeds both
        m-halves back to back. korder="tile": tile-at-a-time k-inner,
        each activation drains during the next tile's accumulation so no
        act backlog hits the following layer (used once DMA is ahead)."""
        if korder == "tile":
            for o in range(OC1):
                for m in range(2):
                    pt = ps.tile([128, MT], F32, tag="ps",
                                 name=f"pt{br}_{m}_{o}")
                    for k in range(KC1):
                        nc.tensor.matmul(
                            pt[:],
                            lhsT=w_sb[:, k * 512 + o * 128:
                                      k * 512 + o * 128 + 128],
                            rhs=x_sb[:, k * NHW + m * MT:k * NHW + m * MT + MT],
                            start=(k == 0), stop=(k == KC1 - 1))
                    col = (br * OC1 + o) * NHW + m * MT
                    nc.scalar.activation(
                        f_sb[:, col:col + MT], pt[:],
                        mybir.ActivationFunctionType.Prelu,
                        bias=bias_sb[:, o:o + 1], scale=1.0, alpha=SLOPE)
            return
        pts = [[ps.tile([128, MT], F32, tag="ps", name=f"pt{br}_{m}_{o}")
                for o in range(OC1)] for m in range(2)]
        for k in range(KC1):
            for o in range(OC1):
                for m in range(2):
                    nc.tensor.matmul(
                        pts[m][o][:],
                        lhsT=w_sb[:, k * 512 + o * 128:k * 512 + o * 128 + 128],
                        rhs=x_sb[:, k * NHW + m * MT:k * NHW + m * MT + MT],
                        start=(k == 0), stop=(k == KC1 - 1))
        for m in range(2):
            for o in range(OC1):
                col = (br * OC1 + o) * NHW + m * MT
                nc.scalar.activation(
                    f_sb[:, col:col + MT], pts[m][o][:],
                    mybir.ActivationFunctionType.Prelu,
                    bias=bias_sb[:, o:o + 1], scale=1.0, alpha=SLOPE)

    m_tiles = []

    def do_l2_l3(b, f_sb, last=False):
        # ---- layer 2: h = lrelu(A1 @ f + a1), fp16 out ----
        # Normal batches: one k-pass over all 4 PSUM tiles (same lhsT
        # feeds both m-halves back to back). Last batch: one m-half per
        # k-pass, so its act -> reduce chain drains while the PE still
        # works the other half and only half the chain sits on the tail.
        h_sb = hp.tile([128, OC2 * NHW], F16, tag="h")
        m_sb = mp.tile([128, OC2 * NN], F32, tag="m", name=f"m_{b}")
        m_v = m_sb.rearrange("p (q n) -> p q n", q=OC2)
        h_v = h_sb.rearrange("p (q n x) -> p q n x", q=OC2, n=NN)
        HN = NN // 2

        def reduce_part(m, qs):
            # layer 3a: reduce the 49 HW positions on the DVE; runs as
            # soon as the covered activations are done
            nc.vector.reduce_sum(
                m_v[:, qs, m * HN:(m + 1) * HN],
                h_v[:, qs, m * HN:(m + 1) * HN, :],
                axis=mybir.AxisListType.X)

        def act_tile(m, o, pt):
            col = o * NHW + m * MT
            nc.scalar.activation(
                h_sb[:, col:col + MT], pt[:],
                mybir.ActivationFunctionType.Prelu,
                bias=b1_sb[:, o:o + 1], scale=1.0, alpha=SLOPE)

        def kpass(tiles_mo):
            pts2 = [ps.tile([128, MT], F32, tag="ps", name=f"pt2_{m}_{o}")
                    for m, o in tiles_mo]
            for k in range(KC2):
                for i, (m, o) in enumerate(tiles_mo):
                    nc.tensor.matmul(
                        pts2[i][:],
                        lhsT=w1_sb[:, k * 256 + o * 128:k * 256 + o * 128 + 128],
                        rhs=f_sb[:, k * NHW + m * MT:k * NHW + m * MT + MT],
                        start=(k == 0), stop=(k == KC2 - 1))
            return pts2

        if not last:
            pts2 = kpass([(m, o) for o in range(OC2) for m in range(2)])
            for i, (m, o) in enumerate([(m, o) for o in range(OC2)
                                        for m in range(2)]):
                act_tile(m, o, pts2[i])
            for m in range(2):
                reduce_part(m, slice(0, OC2))
        else:
            # last batch: one (m,o) tile per k-pass so each act + per-q
            # reduce drains under the next tile's matmuls; the kernel
            # tail carries only the final tile's act + 1-chunk reduce
            for m, o in ((1, 0), (1, 1), (0, 0), (0, 1)):
                pt = kpass([(m, o)])[0]
                act_tile(m, o, pt)
                reduce_part(m, slice(o, o + 1))
        m_tiles.append(m_sb)

    def do_l3(db):
        # layer 3b: y[n] = W2 @ m (tiny fp32 matmuls) + threshold
        # (sigmoid(mean) > 0.5 <=> sum > -49*b2). Batch b's contraction
        # is emitted during a later batch's compute so only the last
        # batch's chain sits on the kernel tail.
        ps3 = ps.tile([1, NN], F32, tag="ps", name=f"ps3_{db}")
        for q in range(OC2):
            nc.tensor.matmul(
                ps3[:], lhsT=w2_sb[:, q:q + 1],
                rhs=m_tiles[db][:, q * NN:(q + 1) * NN],
                start=(q == 0), stop=(q == OC2 - 1))
        nc.vector.tensor_scalar(
            bits_sb[0:1, db * NN:(db + 1) * NN], ps3[:], float(thresh),
            None, mybir.AluOpType.is_gt)

    # ---- batches 0+1 are DMA-critical: run BOTH so-branches first, so
    # the stream prefix is wso + two x_so tensors (wc defers until the
    # PE is ~2 branches in), with weight chunks interleaved right before
    # the k-chunks that consume them. ----
    # finest granules first so the PE's first matmul waits only ~1/4 of
    # wso + one k-chunk of x_so; weight quarter q covers k-chunks 4q..4q+3
    xso0_sb, so0_chunks = load_x(xso_d, 0, [1, 1, 1, 1, 2, 2, 2, 2, 2, 2], "xso")
    xso1_sb, so1_chunks = load_x(xso_d, 1, 4, "xso")
    xc0_sb, c0_chunks = load_x(xc_d, 0, 8, "xc")
    xc1_sb, c1_chunks = load_x(xc_d, 1, 4, "xc")
    WQ = KC1 * 512 // 4
    chained_dma(wso_sb[:, :WQ], wso_d[:, :WQ])           # k0-3
    for sl in so0_chunks[:4]:                            # k0..k3 singly
        chained_dma(*sl)
    chained_dma(wso_sb[:, WQ:2 * WQ], wso_d[:, WQ:2 * WQ])   # k4-7
    for sl in so0_chunks[4:6]:                           # k4,5 / k6,7
        chained_dma(*sl)
    chained_dma(wso_sb[:, 2 * WQ:3 * WQ], wso_d[:, 2 * WQ:3 * WQ])
    for sl in so0_chunks[6:8]:                           # k8,9 / k10,11
        chained_dma(*sl)
    chained_dma(wso_sb[:, 3 * WQ:], wso_d[:, 3 * WQ:])
    for sl in so0_chunks[8:]:                            # k12,13 / k14,15
        chained_dma(*sl)
    for sl in so1_chunks:
        chained_dma(*sl)
    for q in range(4):
        chained_dma(wc_sb[:, q * 2048:(q + 1) * 2048],
                    wc_d[:, q * 2048:(q + 1) * 2048])
        chained_dma(*c0_chunks[2 * q])
        if q == 3:
            chained_dma(w1_sb[:], w1_d[:])
        chained_dma(*c0_chunks[2 * q + 1])
    for sl in c1_chunks:
        chained_dma(*sl)

    f0_sb = fp.tile([128, 2 * OC1 * NHW], F16, tag="f", name="f_0")
    f1_sb = fp.tile([128, 2 * OC1 * NHW], F16, tag="f", name="f_1")
    l1_branch(xso0_sb, wso_sb, bso_sb, f0_sb, 0)
    l1_branch(xso1_sb, wso_sb, bso_sb, f1_sb, 0)
    l1_branch(xc0_sb, wc_sb, bc_sb, f0_sb, 1)
    l1_branch(xc1_sb, wc_sb, bc_sb, f1_sb, 1)
    do_l2_l3(0, f0_sb)
    do_l2_l3(1, f1_sb)
    do_l3(0)

    # ---- batches 2+3: DMA is well ahead; simple per-batch order ----
    for b in (2, 3):
        xso_sb, so_chunks = load_x(xso_d, b, 4, "xso")
        xc_sb, c_chunks = load_x(xc_d, b, 4, "xc")
        for sl in so_chunks:
            chained_dma(*sl)
        for sl in c_chunks:
            chained_dma(*sl)
        f_sb = fp.tile([128, 2 * OC1 * NHW], F16, tag="f", name=f"f_{b}")
        l1_branch(xso_sb, wso_sb, bso_sb, f_sb, 0, korder="tile")
        l1_branch(xc_sb, wc_sb, bc_sb, f_sb, 1, korder="tile")
        do_l2_l3(b, f_sb, last=(b == 3))
        do_l3(b - 1)
    do_l3(3)

    nc.sync.dma_start(out_d[:], bits_sb[0:1, :])


_CACHE = {}


def _get_nc(thresh, loop=1):
    key = (round(thresh, 9), loop)
    if key not in _CACHE:
        _CACHE[key] = build_bass(thresh, loop)
    return _CACHE[key]


def kernel(**inputs):
    wso, wc, w1, w2, bso, bc, b1, thresh = _fold_params(inputs)
    xso = np.ascontiguousarray(
        np.asarray(inputs["x_so"], dtype=np.float32).reshape(B, C, NHW))
    xc = np.ascontiguousarray(
        np.asarray(inputs["x_c"], dtype=np.float32).reshape(B, C, NHW))

    nc = _get_nc(thresh)
    in_maps = []
    for i in range(N_CORES):
        in_maps.append({
            "x_so": xso[i * BPC:(i + 1) * BPC],
            "x_c": xc[i * BPC:(i + 1) * BPC],
            "wso": wso, "wc": wc, "w1": w1, "w2": w2,
            "bso": bso, "bc": bc, "b1": b1,
        })
    res = run_bass_kernel_spmd(nc, in_maps, list(range(N_CORES)))
    out = np.concatenate([res.results[i]["out"].reshape(BPC, NN)
                          for i in range(N_CORES)], axis=0)
    return np.ascontiguousarray(out.reshape(B, NN, 1).astype(np.float32))
